# revision 1
# baseline (speedup 1.0000x reference)
"""Multi-head self-attention (B=1, S=2048, E=1024, H=16, D=64) on 8 NeuronCores.

Tensor-parallel by head: core c owns heads {2c, 2c+1}.  v2 schedule:

  Phase A (DMA window): warm matmuls on memset data keep PE busy (HAM ramp),
    qkv arrives as 16 (k, S-half) slabs striped over 3 DMA queues; per-slab
    priority matmuls accumulate kT chunk0 + qT chunks 0,1 so the score
    stream can start right after the first S-half lands.
  Phase B (t=0..15): scoresT(p0) per head (row-group concurrent K=64 pairs),
    exp on ACT paces the phase; PE filler = remaining in_proj (kT c1-c3,
    qT c2c3, v tiles) + AV chunk c0 trailing.
  Phase C (t=0..15): scoresT(p1) + AV c1 (front-loaded), c2 (trailing),
    c3 (second-half catchup).  exp output for p1 aliases the p0 buffer.
  Tail: recip (ACT, one table load), normalize muls (DVE/GPSIMD),
    out_proj per s-tile, f16 evictions rotated over vector/gpsimd/scalar,
    output DMA striped over 3 queues.

Host sums the 8 partials and adds b_out.
"""

import os
import sys

import numpy as np

try:
    import concourse.bass as bass  # noqa: F401
except ImportError:
    sys.path.insert(0, "/opt/trn_rl_repo")

import ml_dtypes

import concourse.bass as bass
import concourse.mybir as mybir
import concourse.tile as tile
from concourse import bacc, bass_utils

S = 2048
E = 1024
H = 16
D = 64
NCORE = 8
HC = H // NCORE          # heads per core = 2
J = HC * D               # local feature width = 128
KE = E // 128            # contraction tiles for in_proj = 8
NT = S // 128            # 128-row tiles of the sequence = 16
CH = 512                 # s-chunk (one PSUM bank of f32)
NCH = S // CH            # s-chunks = 4
W = 2 * CH               # score tile width (2 chunks) = 1024
SCALE = 1.0 / np.sqrt(D)

MM_DT = mybir.dt.bfloat16
MM_NP = ml_dtypes.bfloat16

_cached = None


def _build():
    f32 = mybir.dt.float32
    f16 = mybir.dt.float16
    nc = bacc.Bacc("TRN2", target_bir_lowering=False, num_swdge_queues=4)

    d_qkvT = nc.dram_tensor("qkvT", (E, S), MM_DT, kind="ExternalInput")
    d_wq = nc.dram_tensor("wq", (128, KE * J), MM_DT, kind="ExternalInput")
    d_wk = nc.dram_tensor("wk", (128, KE * J), MM_DT, kind="ExternalInput")
    d_wv = nc.dram_tensor("wv", (128, KE * J), MM_DT, kind="ExternalInput")
    d_bq = nc.dram_tensor("bq", (J, 1), f32, kind="ExternalInput")
    d_bk = nc.dram_tensor("bk", (J, 1), f32, kind="ExternalInput")
    d_bv = nc.dram_tensor("bv", (128, J), f32, kind="ExternalInput")
    d_wout = nc.dram_tensor("wout", (J, E), MM_DT, kind="ExternalInput")
    d_out = nc.dram_tensor("partial", (S, E), f16, kind="ExternalOutput")

    dq = d_qkvT.rearrange("(k p) m -> p k m", p=128)

    with tile.TileContext(nc) as tc:
        with (
            tc.tile_pool(name="persist", bufs=1) as persist,
            tc.tile_pool(name="outp", bufs=8) as outp,
            tc.tile_pool(name="small", bufs=8) as small,
            # PSUM: 2 x [128,1024] (scores / qT pair / out_proj) = 4 banks
            #       4 x [128,512]  (AV + in_proj accumulators)   = 4 banks
            tc.tile_pool(name="ps_sc", bufs=2, space="PSUM") as ps_sc,
            tc.tile_pool(name="ps_a", bufs=4, space="PSUM") as ps_a,
        ):
            # ---- persistent SBUF ----
            sb_wq = persist.tile([128, KE, J], MM_DT)
            sb_wk = persist.tile([128, KE, J], MM_DT)
            sb_wv = persist.tile([128, KE, J], MM_DT)
            sb_bq = persist.tile([J, 1], f32)
            sb_bk = persist.tile([J, 1], f32)
            sb_bv = persist.tile([128, J], f32)
            sb_wout = persist.tile([J, E], MM_DT)
            sb_qkvT = persist.tile([128, KE, S], MM_DT)
            sb_qT = persist.tile([J, S], MM_DT)
            sb_kT = persist.tile([J, S], MM_DT)
            # v augmented per head with a 64-wide ones block: the AV matmul
            # then yields Z replicated on partitions 64..127
            sb_v = persist.tile([128, NT, HC * 2 * D], MM_DT)
            sb_attnT = persist.tile([J, S], MM_DT)
            # exp buffers, one per head; C-phase aliases the same storage
            ex = [persist.tile([128, NT, W], MM_DT, name=f"ex{h}") for h in range(HC)]
            # raw (un-normalized) AV evictions for chunks 0,1: rows 0:64 out,
            # 64:128 Z, per (chunk, head)
            raw = persist.tile([128, 2, HC, CH], MM_DT)
            sb_warm = persist.tile([128, CH], MM_DT)

            # ---- t0: memsets (vector/gpsimd) ----
            nc.vector.memset(sb_warm[:], 0.125)
            nc.vector.memset(sb_v[:, :, D : 2 * D], 1.0)
            nc.gpsimd.memset(sb_v[:, :, 3 * D :], 1.0)

            # ---- DMA issues ----
            # weights for the priority stream first, then h0 slabs striped,
            # tiny biases, h1 slabs, wout last.
            nc.sync.dma_start(out=sb_wk[:], in_=d_wk[:])
            nc.scalar.dma_start(out=sb_wq[:], in_=d_wq[:])
            nc.gpsimd.dma_start(out=sb_wv[:], in_=d_wv[:])
            qengs = [nc.sync, nc.scalar, nc.gpsimd]
            for k in range(KE - 1):
                qengs[k % 3].dma_start(
                    out=sb_qkvT[:, k, 0:1024], in_=dq[:, k, 0:1024]
                )
            nc.scalar.dma_start(out=sb_qkvT[0:64, 7, 0:1024], in_=dq[0:64, 7, 0:1024])
            nc.gpsimd.dma_start(out=sb_qkvT[64:128, 7, 0:1024], in_=dq[64:128, 7, 0:1024])
            nc.sync.dma_start(out=sb_bq[:], in_=d_bq[:])
            nc.scalar.dma_start(out=sb_bk[:], in_=d_bk[:])
            nc.gpsimd.dma_start(out=sb_bv[:], in_=d_bv[:])
            for k in range(KE):
                qengs[k % 3].dma_start(
                    out=sb_qkvT[:, k, 1024:2048], in_=dq[:, k, 1024:2048]
                )
            nc.sync.dma_start(out=sb_wout[:], in_=d_wout[:])

            # ---- warm matmuls: keep PE busy through the DMA window ----
            warm_ps = [
                ps_sc.tile([128, W], f32, tag="sc", name=f"warm{i}")
                for i in range(2)
            ]
            for i in range(22):
                nc.tensor.matmul(
                    warm_ps[i % 2][:, :CH], sb_warm[:, 0:128], sb_warm[:],
                    start=True, stop=True,
                )

            # ---- phase A: priority in_proj (kT c0, qT c0+c1), k-outer ----
            ps_kc0 = ps_a.tile([128, CH], f32, tag="a", name="kc0")
            ps_q01 = ps_sc.tile([128, W], f32, tag="sc", name="q01")
            for k in range(KE):
                nc.tensor.matmul(
                    ps_kc0[:], sb_wk[:, k, :], sb_qkvT[:, k, 0:CH],
                    start=(k == 0), stop=(k == KE - 1),
                )
                nc.tensor.matmul(
                    ps_q01[:, 0:CH], sb_wq[:, k, :], sb_qkvT[:, k, 0:CH],
                    start=(k == 0), stop=(k == KE - 1),
                )
                nc.tensor.matmul(
                    ps_q01[:, CH:W], sb_wq[:, k, :], sb_qkvT[:, k, CH:W],
                    start=(k == 0), stop=(k == KE - 1),
                )
            nc.vector.tensor_scalar_add(sb_kT[:, 0:CH], ps_kc0[:], sb_bk[:])
            nc.vector.tensor_scalar_add(sb_qT[:, 0:W], ps_q01[:], sb_bq[:])

            # ---- backlog filler units for phase B ----
            # each unit: (emit_fn) issuing ~<=2us of PE work + its add
            add_engs = [nc.gpsimd, nc.vector]

            def mk_vgroup(g, eng):
                def emit():
                    ps_v = ps_a.tile([128, CH], f32, tag="a", name=f"v{g}")
                    for ti in range(4):
                        t = 4 * g + ti
                        for k in range(KE):
                            nc.tensor.matmul(
                                ps_v[:, ti * 128 : (ti + 1) * 128],
                                sb_qkvT[:, k, t * 128 : (t + 1) * 128],
                                sb_wv[:, k, :],
                                start=(k == 0), stop=(k == KE - 1),
                            )
                    # add bias into sb_v (v cols only, skip the ones blocks)
                    for ti in range(4):
                        t = 4 * g + ti
                        for h in range(HC):
                            eng.tensor_add(
                                sb_v[:, t, h * 2 * D : h * 2 * D + D],
                                ps_v[:, ti * 128 + h * D : ti * 128 + (h + 1) * D],
                                sb_bv[:, h * D : (h + 1) * D],
                            )
                return emit

            def mk_kchunk(c, eng):
                def emit():
                    ps_k = ps_a.tile([128, CH], f32, tag="a", name=f"kc{c}")
                    for k in range(KE):
                        nc.tensor.matmul(
                            ps_k[:], sb_wk[:, k, :],
                            sb_qkvT[:, k, c * CH : (c + 1) * CH],
                            start=(k == 0), stop=(k == KE - 1),
                        )
                    eng.tensor_scalar_add(
                        sb_kT[:, c * CH : (c + 1) * CH], ps_k[:], sb_bk[:]
                    )
                return emit

            def mk_qchunk(c, eng):
                def emit():
                    ps_q = ps_a.tile([128, CH], f32, tag="a", name=f"qc{c}")
                    for k in range(KE):
                        nc.tensor.matmul(
                            ps_q[:], sb_wq[:, k, :],
                            sb_qkvT[:, k, c * CH : (c + 1) * CH],
                            start=(k == 0), stop=(k == KE - 1),
                        )
                    eng.tensor_scalar_add(
                        sb_qT[:, c * CH : (c + 1) * CH], ps_q[:], sb_bq[:]
                    )
                return emit

            # emitted at END of B-iteration t (key); consumers:
            # kT chunk c needed by scores at t=4c; v group g by av_c0 per
            # av0_steps; qT c2/c3 by phase C.
            backlog = {
                0: mk_vgroup(0, nc.vector),
                2: mk_kchunk(1, nc.vector),
                4: mk_vgroup(1, nc.vector),
                5: mk_kchunk(2, nc.vector),
                6: mk_vgroup(2, nc.vector),
                9: mk_kchunk(3, nc.vector),
                10: mk_qchunk(2, nc.vector),
                12: mk_vgroup(3, nc.vector),
                13: mk_qchunk(3, nc.vector),
            }

            def scores(t, h, qlo, ps):
                hd = slice(h * D, (h + 1) * D)
                for i in range(2):
                    nc.tensor.matmul(
                        ps[:, i * CH : (i + 1) * CH],
                        sb_kT[hd, t * 128 : (t + 1) * 128],
                        sb_qT[hd, qlo + i * CH : qlo + (i + 1) * CH],
                        start=True, stop=True,
                    )

            def av_step(slot, h, chalf, t, first, last):
                nc.tensor.matmul(
                    slot[:],
                    sb_v[:, t, h * 2 * D : (h + 1) * 2 * D],
                    ex[h][:, t, chalf * CH : (chalf + 1) * CH],
                    start=first, stop=last,
                )

            # ---- phase B: scores p0 + exp; filler: backlog + AV c0 ----
            av0 = [ps_a.tile([128, CH], f32, tag="a", name=f"av0_{h}") for h in range(HC)]
            # av_c0 pacing: 16 steps over t=3..15 -> [2,1,1,1,1,1,1,1,1,1,1,2,2]
            av0_steps = {t: [] for t in range(NT)}
            j = 0
            for t in range(3, NT):
                n = 2 if t in (3, 14, 15) else 1
                for _ in range(n):
                    if j < NT:
                        av0_steps[t].append(j)
                        j += 1
            for t in range(NT):
                sc_ps = [
                    ps_sc.tile([128, W], f32, tag="sc", name=f"scB{t}_{h}")
                    for h in range(HC)
                ]
                for h in range(HC):
                    scores(t, h, 0, sc_ps[h])
                for h in range(HC):
                    nc.scalar.activation(
                        ex[h][:, t, :], sc_ps[h][:],
                        mybir.ActivationFunctionType.Exp, scale=float(SCALE),
                    )
                for jj in av0_steps[t]:
                    for h in range(HC):
                        av_step(av0[h], h, 0, jj, jj == 0, jj == NT - 1)
                if t in backlog:
                    backlog[t]()

            # ---- B seam: evict raw AV c0 (rows 0:64 out, 64:128 Z) ----
            for h in range(HC):
                nc.vector.tensor_copy(raw[:, 0, h, :], av0[h][:])

            # ---- phase C: scores p1 + exp; AV c1 (front), c2, c3 ----
            av1 = [ps_a.tile([128, CH], f32, tag="a", name=f"av1_{h}") for h in range(HC)]
            av2 = [ps_a.tile([128, CH], f32, tag="a", name=f"av2_{h}") for h in range(HC)]
            av3 = None
            # av_c1: 3 steps/t for t=0..4, 1 at t=5 (front-loaded; must stay
            # ahead of the exp aliasing overwrite of ex[:, t, CH:W])
            av1_steps = {t: [] for t in range(NT)}
            j = 0
            for t in range(6):
                for _ in range(3 if t < 5 else 1):
                    if j < NT:
                        av1_steps[t].append(j)
                        j += 1
            for t in range(NT):
                for jj in av1_steps[t]:
                    for h in range(HC):
                        av_step(av1[h], h, 1, jj, jj == 0, jj == NT - 1)
                sc_ps = [
                    ps_sc.tile([128, W], f32, tag="sc", name=f"scC{t}_{h}")
                    for h in range(HC)
                ]
                for h in range(HC):
                    scores(t, h, W, sc_ps[h])
                for h in range(HC):
                    nc.scalar.activation(
                        ex[h][:, t, :], sc_ps[h][:],
                        mybir.ActivationFunctionType.Exp, scale=float(SCALE),
                    )
                # evict av_c1 raw once finished (frees slots for av_c3)
                if t == 6:
                    for h in range(HC):
                        nc.vector.tensor_copy(raw[:, 1, h, :], av1[h][:])
                if t == 8:
                    av3 = [
                        ps_a.tile([128, CH], f32, tag="a", name=f"av3_{h}")
                        for h in range(HC)
                    ]
                # av_c2 trails by 1 tile
                if t >= 1:
                    for h in range(HC):
                        av_step(av2[h], h, 0, t - 1, t - 1 == 0, False)
                # av_c3 catches up 2/t from t=8
                if t >= 8:
                    for jj in (2 * (t - 8), 2 * (t - 8) + 1):
                        for h in range(HC):
                            av_step(av3[h], h, 1, jj, jj == 0, jj == NT - 1)
            for h in range(HC):
                av_step(av2[h], h, 0, NT - 1, False, True)

            # ---- tail: recip + normalize + out_proj + evict + DMA out ----
            def act_recip(out_ap, in_ap):
                eng = nc.scalar
                inst = mybir.InstActivation(
                    name=nc.get_next_instruction_name(),
                    func=mybir.ActivationFunctionType.Reciprocal,
                    ins=[
                        eng.lower_ap(in_ap),
                        mybir.ImmediateValue(dtype=f32, value=0.0),
                        mybir.ImmediateValue(dtype=f32, value=1.0),
                        mybir.ImmediateValue(dtype=f32, value=0.0),
                    ],
                    outs=[eng.lower_ap(out_ap)],
                )
                eng.add_instruction(inst)

            def evict(i, dst, src):
                if i % 2 == 1:
                    nc.scalar.copy(dst, src)
                else:
                    nc.vector.tensor_copy(dst, src)

            warm_t = ps_sc.tile([128, W], f32, tag="sc", name="tailwarm")
            for _ in range(12):
                nc.tensor.matmul(
                    warm_t[:, :CH], sb_warm[:, 0:128], sb_warm[:],
                    start=True, stop=True,
                )

            # ---- lead-in: all recips (ACT) + all muls (vector: PSUM
            # chunks c2/c3; gpsimd: SBUF raw chunks c0/c1) batched first ----
            def psum_srcs(avx, h):
                return avx[h][D : 2 * D, :], avx[h][0:D, :]

            def raw_srcs(ci, h):
                return raw[D : 2 * D, ci, h, :], raw[0:D, ci, h, :]

            chunk_srcs = [
                (2, lambda h: psum_srcs(av2, h)),
                (3, lambda h: psum_srcs(av3, h)),
                (0, lambda h: raw_srcs(0, h)),
                (1, lambda h: raw_srcs(1, h)),
            ]
            for c, get_srcs in chunk_srcs:
                s_sl = slice(c * CH, (c + 1) * CH)
                mul_eng = nc.gpsimd if c in (0, 1) else nc.vector
                for h in range(HC):
                    hd = slice(h * D, (h + 1) * D)
                    z_ap, o_ap = get_srcs(h)
                    rbc = small.tile([D, CH], f32, tag="rbc", name=f"rbc{c}_{h}")
                    act_recip(rbc[:], z_ap)
                    mul_eng.tensor_mul(sb_attnT[hd, s_sl], o_ap, rbc[:])

            # ---- out_proj stream: pure evicts on vector/scalar, DMA on
            # sync/gpsimd/scalar rotation ----
            # evict: 10 vector / 6 scalar; DMA: sync 6 / gpsimd 6 / scalar 4
            evict_scalar = {1, 4, 7, 10, 12, 14}
            dma_rot = [nc.sync, nc.gpsimd, nc.sync, nc.gpsimd, nc.scalar,
                       nc.sync, nc.gpsimd, nc.sync, nc.gpsimd, nc.scalar,
                       nc.sync, nc.gpsimd, nc.sync, nc.gpsimd, nc.scalar,
                       nc.scalar]
            di = 0
            for c in (2, 3, 0, 1):
                for st in range(CH // 128):
                    t = c * (CH // 128) + st
                    ps_p = ps_sc.tile([128, W], f32, tag="sc", name=f"op{t}")
                    for ec in range(2):
                        nc.tensor.matmul(
                            ps_p[:, ec * CH : (ec + 1) * CH],
                            sb_attnT[:, t * 128 : (t + 1) * 128],
                            sb_wout[:, ec * CH : (ec + 1) * CH],
                            start=True, stop=True,
                        )
                    sb_out = outp.tile([128, E], f16, tag="out", name=f"o{t}")
                    if di in evict_scalar:
                        nc.scalar.copy(sb_out[:], ps_p[:])
                    else:
                        nc.vector.tensor_copy(sb_out[:], ps_p[:])
                    dma_rot[di].dma_start(
                        out=d_out[t * 128 : (t + 1) * 128, :], in_=sb_out[:]
                    )
                    di += 1

    nc.finalize()
    return nc


def _pack_w(w):
    # [E, J] -> [128, KE*J] in (p, k, m) order for a contiguous-row DMA
    return np.ascontiguousarray(
        np.asarray(w, np.float32).reshape(KE, 128, J).transpose(1, 0, 2).reshape(128, KE * J)
    ).astype(MM_NP)


def _prep_inputs(qkv, w_in, b_in, w_out):
    qkv2 = np.asarray(qkv, np.float32).reshape(S, E)
    qkvT = np.ascontiguousarray(qkv2.T).astype(MM_NP)
    w_in = np.asarray(w_in, np.float32)
    b_in = np.asarray(b_in, np.float32)
    w_out = np.asarray(w_out, np.float32)
    in_maps = []
    for c in range(NCORE):
        cols = slice(c * J, c * J + J)
        in_maps.append(
            {
                "qkvT": qkvT,
                "wq": _pack_w(w_in[:, :E][:, cols]),
                "wk": _pack_w(w_in[:, E : 2 * E][:, cols]),
                "wv": _pack_w(w_in[:, 2 * E :][:, cols]),
                "bq": np.ascontiguousarray(b_in[:E][cols]).reshape(J, 1),
                "bk": np.ascontiguousarray(b_in[E : 2 * E][cols]).reshape(J, 1),
                "bv": np.broadcast_to(
                    b_in[2 * E :][cols].reshape(1, J), (128, J)
                ).copy(),
                "wout": np.ascontiguousarray(w_out[cols, :]).astype(MM_NP),
            }
        )
    return in_maps


def kernel(qkv, w_in, b_in, w_out, b_out, _trace=False):
    global _cached
    if _cached is None:
        _cached = _build()
    nc = _cached
    in_maps = _prep_inputs(qkv, w_in, b_in, w_out)
    res = bass_utils.run_bass_kernel_spmd(
        nc, in_maps, core_ids=list(range(NCORE)), trace=_trace
    )
    acc = np.zeros((S, E), np.float64)
    for r in res.results:
        acc += r["partial"].astype(np.float64)
    out = (acc + np.asarray(b_out, np.float32)[None, :]).astype(np.float32)
    out = out.reshape(1, S, E)
    if _trace:
        kernel.last_exec_time_ns = res.exec_time_ns
    return out

